# revision 1
# baseline (speedup 1.0000x reference)
"""Depthwise 7x7 conv (stride 1, pad 3) on 8 NeuronCores via Bass.

Strategy: channel-sharded SPMD (48 channels/core).  Per channel, conv along H
is a banded matmul on TensorE (stationary = banded filter matrix G, moving =
X rows); the 7 kw taps accumulate in PSUM via free-dim-shifted rhs slices.
Matmuls run in float32r (~fp22 multiply, fp32 accumulate) at full PE rate.
Banded matrices are precomputed on host and DMA'd per channel.

H tiling is uniform (stride 122) over host-padded X (3 zero rows on top) so
each channel needs 2 input DMAs (4 merged 128-row overlapping windows + 1
runt) and 2 output DMAs (4 merged 122-row windows + 1 runt).  Loads go on
the SP HWDGE ring, stores on the ACT ring.
"""

import numpy as np

import concourse.bacc as bacc
import concourse.mybir as mybir
import concourse.tile as tile
from concourse.ap import AP
from concourse.bass_utils import run_bass_kernel_spmd

C, H, W_DIM = 384, 512, 512
KH = KW = 7
PAD = 3
N_CORES = 8
CPC = C // N_CORES  # 48 channels per core

GW = 125   # master banded-matrix width
HP = 520   # padded rows per channel on host (3 zero top + 512 + 5 zero tail)
MT = 122   # uniform output rows per full tile
NFULL = 4  # full tiles per channel
MR = H - NFULL * MT  # runt output rows (24)
KR = MR + PAD        # runt contraction rows (27)
XW = W_DIM  # per-window SBUF width (margin-free)
YP = 640  # padded output rows: stores write full 128 partitions (tail junk)

import os as _os
STORE_ENG = lambda nc: nc.scalar  # stores on ACT HWDGE ring, loads on SP
G_ENG = lambda nc: nc.sync
N_XBUF = int(_os.environ.get("N_XBUF", "8"))
N_OBUF = int(_os.environ.get("N_OBUF", "12"))
RUNT_PACK = _os.environ.get("RUNT_PACK", "1") == "1"

F32 = mybir.dt.float32
F32R = mybir.dt.float16  # input dtype (fp16: half DMA bytes, ~fp22 multiply)
NP_IN = np.float16


def _ap(base, dims):
    return AP(tensor=base.tensor, offset=base.offset, ap=list(dims))


def emit_body(nc, g_pool, ps_pool, o_ts, x_ts, x_dram, g_dram, y_dram,
              cpc, ti0=0, skip_loads=False, skip_stores=False, g_static=None):
    """One pass over `cpc` channels (groups of 4 share packed runt matmuls)."""
    w = W_DIM
    ti = ti0
    oi = 0
    runt_state = []
    for c in range(cpc):
        if skip_loads:
            g_t = g_static
        else:
            g_t = g_pool.tile([128, KW * GW], F32R, tag="g", name="g_t")
            G_ENG(nc).dma_start(g_t[:], g_dram[c])
        x_t = x_ts[ti % len(x_ts)]
        ti += 1
        if not skip_loads:
            # 4 overlapping 128-row windows (padded rows 122t..122t+127)
            src4 = _ap(x_dram[c], [[w, 128], [MT * w, NFULL], [1, w]])
            nc.sync.dma_start(x_t[:, 0 : NFULL * w], src4)
            # runt window: padded rows 488..514 (27 rows); in packed mode
            # it lands at partition offset 32*(c%4) of the channel's tile
            r_off = 32 * (c % 4) if RUNT_PACK else 0
            nc.sync.dma_start(
                x_t[r_off : r_off + KR, NFULL * XW : NFULL * XW + w],
                x_dram[c, NFULL * MT : NFULL * MT + KR, :],
            )

        for t in range(NFULL):
            ps_t = ps_pool.tile([128, w], F32, tag="ps", name="ps_t")
            # kw=PAD (shift 0) first: full-width start=True sets has_written
            # for the whole bank; shifted kws accumulate clipped subranges.
            kws = [PAD] + [k for k in range(KW) if k != PAD]
            for idx, kw in enumerate(kws):
                s = kw - PAD
                w_lo = max(0, -s)
                w_hi = w + min(0, -s)
                lhs = g_t[:128, kw * GW + PAD : kw * GW + PAD + MT]
                rhs = x_t[:128, t * XW + w_lo + s : t * XW + w_hi + s]
                nc.tensor.matmul(
                    ps_t[:MT, w_lo:w_hi], lhs, rhs,
                    start=(idx == 0), stop=(idx == KW - 1),
                )
            o_t = o_ts[oi % len(o_ts)]
            oi += 1
            nc.vector.tensor_copy(o_t[:MT, :], ps_t[:MT, :])
            if not skip_stores:
                # full-128-partition store; rows MT..127 are junk that the
                # next window's store overwrites (same ring, WAW order)
                STORE_ENG(nc).dma_start(
                    y_dram[c, t * MT : t * MT + 128, :], o_t[:, :])
        if not RUNT_PACK:
            ps_r = ps_pool.tile([128, w], F32, tag="ps", name="ps_r")
            kws = [PAD] + [k for k in range(KW) if k != PAD]
            for idx, kw in enumerate(kws):
                s = kw - PAD
                w_lo = max(0, -s)
                w_hi = w + min(0, -s)
                lhs = g_t[:KR, kw * GW + PAD : kw * GW + PAD + MR]
                rhs = x_t[:KR, NFULL * XW + w_lo + s : NFULL * XW + w_hi + s]
                nc.tensor.matmul(
                    ps_r[:MR, w_lo:w_hi], lhs, rhs,
                    start=(idx == 0), stop=(idx == KW - 1),
                )
            o_r = o_ts[oi % len(o_ts)]
            oi += 1
            nc.vector.tensor_copy(o_r[:MR, :], ps_r[:MR, :])
            if not skip_stores:
                STORE_ENG(nc).dma_start(
                    y_dram[c, NFULL * MT : NFULL * MT + 128, :], o_r[:, :]
                )
        else:
            runt_state.append((c, g_t, x_t))
            if len(runt_state) == 4:
                o_r = o_ts[oi % len(o_ts)]
                oi += 1
                for i, (ci, g_i, x_i) in enumerate(runt_state):
                    ps_r = ps_pool.tile([128, w], F32, tag="ps", name="ps_r")
                    kws = [PAD] + [k for k in range(KW) if k != PAD]
                    for idx, kw in enumerate(kws):
                        s = kw - PAD
                        w_lo = max(0, -s)
                        w_hi = w + min(0, -s)
                        lhs = g_i[32 * i : 32 * i + KR,
                                  kw * GW + PAD + 32 * i :
                                  kw * GW + PAD + 32 * i + MR]
                        rhs = x_i[32 * i : 32 * i + KR,
                                  NFULL * XW + w_lo + s : NFULL * XW + w_hi + s]
                        nc.tensor.matmul(
                            ps_r[32 * i : 32 * i + MR, w_lo:w_hi], lhs, rhs,
                            start=(idx == 0), stop=(idx == KW - 1),
                            tile_position=(32 * i, 32 * i),
                        )
                    nc.vector.tensor_copy(o_r[32 * i : 32 * i + MR, :],
                                          ps_r[32 * i : 32 * i + MR, :])
                if not skip_stores:
                    g0 = runt_state[0][0]
                    base = y_dram[g0]
                    dst = AP(tensor=base.tensor,
                             offset=base.offset + NFULL * MT * w,
                             ap=[[YP * w, 4], [w, 32], [1, w]])
                    STORE_ENG(nc).dma_start(dst, o_r[:, :])
                runt_state.clear()
    return ti


def build_nc(cpc=CPC, n_xbuf=None):
    n_xbuf = n_xbuf or N_XBUF
    w = W_DIM
    nc = bacc.Bacc(None, target_bir_lowering=False)

    x_dram = nc.dram_tensor("X", [cpc, HP, w], F32R, kind="ExternalInput")
    g_dram = nc.dram_tensor("G", [cpc, 128, KW, GW], F32R, kind="ExternalInput")
    y_dram = nc.dram_tensor("Y", [cpc, YP, w], F32, kind="ExternalOutput")

    with tile.TileContext(nc) as tc:
        with (
            tc.tile_pool(name="xw", bufs=1) as x_pool,
            tc.tile_pool(name="g", bufs=6) as g_pool,
            tc.tile_pool(name="ps", bufs=6, space="PSUM") as ps_pool,
            tc.tile_pool(name="ob", bufs=1) as o_pool,
        ):
            x_ts = [
                x_pool.tile([128, 5 * w], F32R, tag=f"x{i}", name=f"x{i}")
                for i in range(n_xbuf)
            ]

            o_ts = [
                o_pool.tile([128, w], F32, tag=f"o{i}", name=f"o{i}")
                for i in range(N_OBUF)
            ]
            for o_t in o_ts:
                nc.vector.memset(o_t[:, :], 0.0)
            emit_body(nc, g_pool, ps_pool, o_ts, x_ts,
                      x_dram, g_dram, y_dram, cpc)

    nc.compile()
    return nc


def build_g(wf):
    """wf: (C, 7, 7) filters -> (C, 128, 7, GW) float32 banded matrices.

    G[c, j, kw, m2] = wf[c, j - m2 + 3, kw] where valid (0..6), else 0.
    Every tile slices at g_off=PAD: lhsT[j, h] = wf[j - h] with padded input.
    """
    c = wf.shape[0]
    g = np.zeros((c, 128, KW, GW), dtype=NP_IN)
    js = np.arange(128)
    for kh in range(KH):
        m2 = js + 3 - kh
        mask = (m2 >= 0) & (m2 < GW)
        g[:, js[mask], :, m2[mask]] = wf[None, :, kh, :].astype(NP_IN)
    return g


def pad_x(x):
    """(C, H, W) -> (C, HP, W) fp16 with 3 zero rows on top, zero tail."""
    c, h, w = x.shape
    xp = np.zeros((c, HP, w), dtype=NP_IN)
    xp[:, PAD : PAD + h] = x.astype(NP_IN)
    return xp


_NC_CACHE = {}


def _get_nc():
    if CPC not in _NC_CACHE:
        _NC_CACHE[CPC] = build_nc(CPC)
    return _NC_CACHE[CPC]


def run(X, W, **spmd_kwargs):
    X = np.asarray(X, dtype=np.float32)
    W = np.asarray(W, dtype=np.float32)
    wf = np.ascontiguousarray(W[:, 0])  # (C, 7, 7)
    g_all = build_g(wf)
    xp = pad_x(X)

    nc = _get_nc()
    in_maps = []
    for core in range(N_CORES):
        c0 = core * CPC
        in_maps.append(
            {
                "X": np.ascontiguousarray(xp[c0 : c0 + CPC]),
                "G": np.ascontiguousarray(g_all[c0 : c0 + CPC]),
            }
        )
    res = run_bass_kernel_spmd(nc, in_maps, core_ids=list(range(N_CORES)),
                               **spmd_kwargs)
    y = np.concatenate([r["Y"][:, :H, :] for r in res.results], axis=0)
    return y, res


def kernel(X, W):
    return run(X, W)[0]

